# revision 1
# baseline (speedup 1.0000x reference)
"""ContextCluster (denoised) Trainium2 kernel — 8-core SPMD.

Sharding: core c handles batch b=c//4, spatial w-quarter q=c%4
(8 of 32 w-planes => 8192 of 32768 points, all 4 heads).
Centers (adaptive-pooled x) and per-center aggregation sums are combined
across the 4 cores of each batch with two tiny collectives
(AllGather of pooled-x, AllReduce of per-center sums).

Math layout choices (per core, N=8192 points, M=64 centers, 4 heads x 24 ch):
 - feat conv in fp32 NAT layout [96c, N]  (argmax-critical -> exact fp32)
 - sim u = chat^T @ feat as PE matmul -> psum [128n, 256(h,m)] fp32
 - q = free-axis reduce_max; mask = (u == q) (exact, no fp32 ties)
 - everything downstream (value, masked weights, dispatch) in fp16:
   measured end-to-end rel err ~3e-4 vs fp32 reference
 - dispatch gather = PE matmul with DMA-transposed fp16 masks
"""

import sys

sys.path.insert(0, '/opt/trn_rl_repo')

import numpy as np

import concourse.bass as bass
import concourse.bacc as bacc
import concourse.tile as tile
from concourse import mybir
from concourse.bass_utils import run_bass_kernel_spmd

F32 = mybir.dt.float32
F16 = mybir.dt.float16

N_CORES = 8
B, CIN, S = 2, 64, 32          # x: [B, CIN, S, S, S]
HEADS, HD = 4, 24              # heads, head_dim
COUT = HEADS * HD              # 96
PROP = 4
M = PROP ** 3                  # 64 centers
MLOC = 16                      # centers owned per core (one w-quarter)
NP = 8 * S * S                 # 8192 points per core
NCH = NP // 128                # 64 chunks of 128 points
GROUPS = [[0, 1, 2, 3], [4, 5, 6, 7]]

_CACHE = {}


def build(debug=False):
    key = ("nc", debug)
    if key in _CACHE:
        return _CACHE[key]
    nc = bacc.Bacc("TRN2", target_bir_lowering=False, debug=False,
                   num_devices=N_CORES)

    dram_in = {}
    def din(name, shape, dt):
        dram_in[name] = nc.dram_tensor(name, shape, dt, kind="ExternalInput")
        return dram_in[name]

    xs    = din("xs",    [CIN + 1, NP], F32)   # x slice + ones row
    xs16  = din("xs16",  [CIN + 1, NP], F16)
    wft   = din("wft",   [CIN + 1, COUT], F32) # [Wf.T; bf]
    wvt   = din("wvt",   [CIN + 1, HEADS * (HD + 1)], F16) # per-head [Wv_h.T|1; bv_h|1]
    wvt32 = din("wvt32", [CIN, COUT], F32)     # Wv.T (for v_centers)
    wpt4  = din("wpt4",  [HD, HEADS * CIN], F32)  # wpt4[c, h*64+o] = Wp[o, 24h+c]
    hsel16 = din("hsel16", [COUT, HEADS], F16)    # head one-hot
    hsel32 = din("hsel32", [COUT, HEADS], F32)
    hselr  = din("hselr",  [HEADS, COUT], F32)    # transposed one-hot
    bfrow  = din("bfrow",  [1, COUT], F32)
    bvrow  = din("bvrow",  [1, COUT], F32)
    ones64 = din("ones64", [1, M], F32)
    ones512 = din("ones512", [1, 512], F16)
    bprow16 = din("bprow16", [1, CIN], F16)
    sgn4   = din("sgn4",   [HEADS, 1], F32)       # sign(alpha)
    abcol  = din("abcol",  [128, 2], F32)         # |alpha|, beta

    y_out = nc.dram_tensor("y_out", [CIN, NP], F32, kind="ExternalOutput")
    if debug:
        dbg = {}
        def dout(name, shape, dt):
            dbg[name] = nc.dram_tensor(name, shape, dt, kind="ExternalOutput")
            return dbg[name]
        d_px = dout("d_px", [CIN + 1, M], F32)
        d_chat = dout("d_chat", [COUT, M], F32)
        d_q = dout("d_q", [128, NCH, HEADS], F32)
        d_inv = dout("d_inv", [128, NCH, HEADS], F32)
        d_s = dout("d_s", [128, NCH, HEADS], F32)
        d_ms = dout("d_ms", [128, NCH, HEADS * M], F16)
        d_mn0 = dout("d_mn0", [128, NCH, 128], F16)
        d_asf = dout("d_asf", [HEADS * (HD + 1), HEADS * M], F32)
        d_asar = dout("d_asar", [HEADS, HD + 1, M], F32)
        d_g20 = dout("d_g20", [128, CIN], F16)
        d_g21 = dout("d_g21", [128, CIN], F16)
        d_f = dout("d_f", [COUT, NP], F32)
        d_v = dout("d_v", [128, NCH, HEADS * (HD + 1)], F16)
        d_dcol = dout("d_dcol", [2, 128, 1], F32)
        d_num = dout("d_num", [HEADS, HD, M], F32)
        d_sv = dout("d_sv", [128, NCH, HEADS * (HD + 1)], F16)  # now == V

    with tile.TileContext(nc) as tc:
        with tc.tile_pool(name="consts", bufs=1) as consts, \
             tc.tile_pool(name="xp", bufs=1) as xp, \
             tc.tile_pool(name="fp", bufs=1) as fp, \
             tc.tile_pool(name="vp", bufs=1) as vp, \
             tc.tile_pool(name="mp", bufs=1) as mp, \
             tc.tile_pool(name="sm", bufs=1) as sm, \
             tc.tile_pool(name="ystg", bufs=3) as ystg_pool, \
             tc.tile_pool(name="psBIG", bufs=3, space="PSUM") as psBIG, \
             tc.tile_pool(name="psMID", bufs=2, space="PSUM") as psMID, \
             tc.tile_pool(name="psL", bufs=1, space="PSUM") as psL_pool, \
             tc.tile_pool(name="psS", bufs=2, space="PSUM") as psS_pool, \
             tc.tile_pool(name="dram", bufs=1, space="DRAM") as dram:

            # ---------------- const loads ----------------
            c_wft = consts.tile([CIN + 1, COUT], F32)
            c_wvt = consts.tile([CIN + 1, HEADS * (HD + 1)], F16)
            c_wvt32 = consts.tile([CIN, COUT], F32)
            c_wpt4 = consts.tile([HD, HEADS * CIN], F32)
            c_h16 = consts.tile([COUT, HEADS], F16)
            c_h32 = consts.tile([COUT, HEADS], F32)
            c_hr = consts.tile([HEADS, COUT], F32)
            c_bf = consts.tile([1, COUT], F32)
            c_bv = consts.tile([1, COUT], F32)
            c_o64 = consts.tile([1, M], F32)
            c_o512 = consts.tile([1, 512], F16)
            c_bp = consts.tile([1, CIN], F16)
            c_sgn = consts.tile([HEADS, 1], F32)
            c_ab = consts.tile([128, 2], F32)
            for t, d in [(c_wft, wft), (c_wvt, wvt), (c_wvt32, wvt32),
                         (c_wpt4, wpt4), (c_h16, hsel16), (c_h32, hsel32),
                         (c_hr, hselr), (c_bf, bfrow), (c_bv, bvrow),
                         (c_o64, ones64), (c_o512, ones512), (c_bp, bprow16),
                         (c_sgn, sgn4), (c_ab, abcol)]:
                nc.sync.dma_start(t[:], d[:])

            # ---------------- input loads ----------------
            X = xp.tile([CIN + 1, NP], F32, tag="bigA")
            X16 = xp.tile([CIN + 1, NP], F16, tag="bigB")
            nc.sync.dma_start(X[:], xs[:])
            nc.sync.dma_start(X16[:], xs16[:])

            # ---------------- pooling of x -> local centers ----------------
            # free layout: n = w*1024 + (hb*8+h')*32 + db*8 + d'
            P1t = sm.tile([CIN, 1024], F32)
            nc.vector.reduce_sum(P1t[:], X[0:CIN, :].rearrange(
                "c (a d) -> c a d", d=8), axis=mybir.AxisListType.X)
            P2t = sm.tile([CIN, 128], F32)
            nc.vector.reduce_sum(
                P2t[:].rearrange("c (a db) -> c a db", db=4),
                P1t[:].rearrange("c (a hp db) -> c a db hp", hp=8, db=4),
                axis=mybir.AxisListType.X)
            PXL = sm.tile([CIN, MLOC], F32)
            nc.vector.reduce_sum(
                PXL[:].rearrange("c (hb db) -> c hb db", db=4),
                P2t[:].rearrange("c (w hb db) -> c hb db w", w=8, db=4),
                axis=mybir.AxisListType.X)
            nc.vector.tensor_scalar_mul(PXL[:], PXL[:], 1.0 / 512.0)

            px_loc = dram.tile([CIN, MLOC], F32)
            px_g = dram.tile([4, CIN, MLOC], F32)
            nc.sync.dma_start(px_loc[:], PXL[:])
            nc.gpsimd.collective_compute(
                "AllGather", mybir.AluOpType.bypass,
                replica_groups=GROUPS,
                ins=[px_loc.opt()], outs=[px_g.opt()])
            PXS = sm.tile([CIN + 1, M], F32)   # pooled x (all 64 centers) + ones row
            nc.sync.dma_start(PXS[0:CIN, :].rearrange("c (r m) -> c r m", r=4),
                              px_g[:].rearrange("r c m -> c r m"))
            nc.sync.dma_start(PXS[CIN:CIN + 1, :], ones64[:])

            if debug:
                nc.sync.dma_start(d_px[:], PXS[:])

            # ---------------- value conv (fp16, T layout) ----------------
            # per-head columns [v_h | 1]; the ones column comes from the
            # bias row of X16 times the 1 entries in wvt's bias row
            VW = HEADS * (HD + 1)  # 100
            V = vp.tile([128, NCH, VW], F16)
            for g in range(16):  # 4 chunks per psum bank
                psv = psMID.tile([128, 4 * VW], F32, tag="pvy")
                for k in range(4):
                    j = 4 * g + k
                    nc.tensor.matmul(psv[:, k * VW:(k + 1) * VW],
                                     X16[:, j * 128:(j + 1) * 128],
                                     c_wvt[:], start=True, stop=True)
                nc.scalar.copy(V[:, 4 * g:4 * g + 4, :].rearrange("p a c -> p (a c)"),
                               psv[:])

            # ---------------- feat conv (fp32, NAT layout) ----------------
            F = fp.tile([COUT, NP], F32)
            FSQ = fp.tile([COUT, NP], F16)
            for t in range(16):
                psf = psBIG.tile([COUT, 512], F32, tag="pfu")
                nc.tensor.matmul(psf[:], c_wft[:], X[:, t * 512:(t + 1) * 512],
                                 start=True, stop=True)
                nc.scalar.copy(F[:, t * 512:(t + 1) * 512], psf[:])
                nc.scalar.square(FSQ[:, t * 512:(t + 1) * 512], psf[:])

            # ---------------- per-point sum-of-squares -> 1/||x|| ----------------
            psq = psL_pool.tile([128, NCH * HEADS], F32, tag="pl")
            for j in range(NCH):
                nc.tensor.matmul(psq[:, j * HEADS:(j + 1) * HEADS],
                                 FSQ[:, j * 128:(j + 1) * 128],
                                 c_h16[:], start=True, stop=True)
            SQPT = sm.tile([128, NCH, HEADS], F32)
            nc.vector.tensor_copy(SQPT[:].rearrange("p a c -> p (a c)"), psq[:])
            INVPT = sm.tile([128, NCH, HEADS], F32)
            nc.scalar.sqrt(INVPT[:], SQPT[:])
            nc.vector.tensor_scalar_max(INVPT[:], INVPT[:], 1e-12)
            nc.vector.reciprocal(INVPT[:], INVPT[:])

            # ---------------- centers ----------------
            pscf = psS_pool.tile([COUT, M], F32, tag="ps")
            nc.tensor.matmul(pscf[:], c_wft[:], PXS[:], start=True, stop=True)
            SQC = sm.tile([COUT, M], F32)
            nc.scalar.square(SQC[:], pscf[:])
            CFS = sm.tile([COUT, M], F32)
            nc.scalar.copy(CFS[:], pscf[:])
            pshs = psS_pool.tile([HEADS, M], F32, tag="ps")
            nc.tensor.matmul(pshs[:], c_h32[:], SQC[:], start=True, stop=True)
            INV4 = sm.tile([HEADS, M], F32)
            nc.scalar.sqrt(INV4[:], pshs[:])
            nc.vector.tensor_scalar_max(INV4[:], INV4[:], 1e-12)
            nc.vector.reciprocal(INV4[:], INV4[:])
            nc.vector.tensor_scalar(out=INV4[:], in0=INV4[:],
                                    scalar1=c_sgn[:, 0:1], scalar2=None,
                                    op0=mybir.AluOpType.mult)
            psie = psS_pool.tile([COUT, M], F32, tag="ps")
            nc.tensor.matmul(psie[:], c_hr[:], INV4[:], start=True, stop=True)
            CHAT = sm.tile([COUT, M], F32)
            nc.vector.tensor_tensor(out=CHAT[:], in0=CFS[:], in1=psie[:],
                                    op=mybir.AluOpType.mult)
            CBLK = sm.tile([COUT, HEADS * M], F32)
            nc.vector.memset(CBLK[:], 0.0)
            for h in range(HEADS):
                nc.sync.dma_start(CBLK[h * HD:(h + 1) * HD, h * M:(h + 1) * M],
                                  CHAT[h * HD:(h + 1) * HD, :])

            if debug:
                nc.sync.dma_start(d_chat[:], CHAT[:])

            # v_centers (fp32): vc_h = Wv_h @ px + bv_h
            VCs = []
            for h in range(HEADS):
                psvc = psS_pool.tile([HD, M], F32, tag="ps")
                nc.tensor.matmul(psvc[:], c_wvt32[:, h * HD:(h + 1) * HD],
                                 PXS[0:CIN, :], start=True, stop=False)
                nc.tensor.matmul(psvc[:], c_bv[:, h * HD:(h + 1) * HD],
                                 c_o64[:], start=False, stop=True)
                vc = sm.tile([HD, M], F32, tag="vc" + str(h))
                nc.scalar.copy(vc[:], psvc[:])
                VCs.append(vc)

            # ---------------- sim + argmax mask ----------------
            Q = sm.tile([128, NCH, HEADS], F32)
            MS = mp.tile([128, NCH, HEADS * M], F16)
            for t in range(32):          # 32 psum banks x 2 chunks
                psu = psBIG.tile([128, 512], F32, tag="pfu")
                for k in range(2):
                    j = 2 * t + k
                    nc.tensor.matmul(psu[:, k * 256:(k + 1) * 256],
                                     F[:, j * 128:(j + 1) * 128],
                                     CBLK[:], start=True, stop=True)
                nc.vector.reduce_max(
                    Q[:, 2 * t:2 * t + 2, :],
                    psu[:].rearrange("p (j h m) -> p j h m", j=2, h=HEADS),
                    axis=mybir.AxisListType.X)
                nc.vector.tensor_tensor(
                    out=MS[:, 2 * t:2 * t + 2, :].rearrange(
                        "p j (h m) -> p j h m", h=HEADS),
                    in0=psu[:].rearrange("p (j h m) -> p j h m", j=2, h=HEADS),
                    in1=Q[:, 2 * t:2 * t + 2, :, None].broadcast_to(
                        [128, 2, HEADS, M]),
                    op=mybir.AluOpType.is_equal)

            # ---------------- s = sigmoid(beta + alpha * q / ||x||) ----------------
            Z = sm.tile([128, NCH, HEADS], F32)
            nc.vector.tensor_tensor(out=Z[:], in0=Q[:], in1=INVPT[:],
                                    op=mybir.AluOpType.mult)
            Ssig = sm.tile([128, NCH, HEADS], F32)
            nc.scalar.activation(Ssig[:].rearrange("p a c -> p (a c)"),
                                 Z[:].rearrange("p a c -> p (a c)"),
                                 mybir.ActivationFunctionType.Sigmoid,
                                 bias=c_ab[:, 1:2], scale=c_ab[:, 0:1])

            if debug:
                nc.sync.dma_start(d_q[:], Q[:])
                nc.sync.dma_start(d_inv[:], INVPT[:])
                nc.sync.dma_start(d_s[:], Ssig[:])
                nc.sync.dma_start(d_f[:], F[:])
                nc.sync.dma_start(d_v[:], V[:])

            # masked = mask * s ; SV = [value*s | s] per head
            for half in range(2):
                js = slice(32 * half, 32 * (half + 1))
                nc.vector.tensor_tensor(
                    out=MS[:, js, :].rearrange("p j (h m) -> p j h m", h=HEADS),
                    in0=MS[:, js, :].rearrange("p j (h m) -> p j h m", h=HEADS),
                    in1=Ssig[:, js, :, None].broadcast_to([128, 32, HEADS, M]),
                    op=mybir.AluOpType.mult)

            # ---------------- per-center sums (vsum & count) ----------------
            psas = psL_pool.tile([HEADS * (HD + 1), HEADS * M], F32, tag="pl")
            for j in range(NCH):
                nc.tensor.matmul(psas[:], V[:, j, :], MS[:, j, :],
                                 start=(j == 0), stop=(j == NCH - 1))
            ASF = sm.tile([HEADS * (HD + 1), HEADS * M], F32)
            nc.scalar.copy(ASF[:], psas[:])
            as_loc = dram.tile([HEADS, HD + 1, M], F32)
            as_ar = dram.tile([HEADS, HD + 1, M], F32)
            for h in range(HEADS):
                nc.sync.dma_start(
                    as_loc[h],
                    ASF[h * (HD + 1):(h + 1) * (HD + 1), h * M:(h + 1) * M])
            nc.gpsimd.collective_compute(
                "AllReduce", mybir.AluOpType.add,
                replica_groups=GROUPS,
                ins=[as_loc.opt()], outs=[as_ar.opt()])

            # ---------------- mask transposes (fp16 DMA-transpose) ----------------
            # reuse X/X16 slots (inputs are fully consumed by this point)
            MN0 = xp.tile([128, NCH, 128], F16, tag="bigA")
            MN1 = xp.tile([128, NCH, 128], F16, tag="bigB")
            for j in range(NCH):
                nc.sync.dma_start(MN0[:, j, :], MS[:, j, 0:128], transpose=True)
                nc.sync.dma_start(MN1[:, j, :], MS[:, j, 128:256], transpose=True)

            if debug:
                nc.sync.dma_start(d_sv[:], V[:])
                nc.sync.dma_start(d_ms[:], MS[:])
                nc.sync.dma_start(d_mn0[:], MN0[:])
                nc.sync.dma_start(d_asf[:], ASF[:])
                nc.sync.dma_start(d_asar[:], as_ar[:])

            # ---------------- agg -> G2 = d * (vsum+vc) @ WpT ----------------
            G2P = []
            for p in range(2):
                psg = psS_pool.tile([128, CIN], F32, tag="ps")
                for hh in range(2):
                    h = 2 * p + hh
                    ASh = sm.tile([HD + 1, M], F32, tag="ash")
                    nc.sync.dma_start(ASh[:], as_ar[h])
                    NUM = sm.tile([HD, M], F32, tag="numh")
                    nc.vector.tensor_tensor(out=NUM[:], in0=ASh[0:HD, :],
                                            in1=VCs[h][:],
                                            op=mybir.AluOpType.add)
                    if debug:
                        nc.sync.dma_start(d_num[h], NUM[:])
                    nc.tensor.matmul(psg[64 * hh:64 * (hh + 1), :], NUM[:],
                                     c_wpt4[:, h * CIN:(h + 1) * CIN],
                                     start=True, stop=True)
                dcol = sm.tile([128, 1], F32, tag="dcol")
                nc.sync.dma_start(
                    dcol[0:64, :],
                    as_ar[2 * p, HD:HD + 1, :].rearrange("c m -> m c"))
                nc.sync.dma_start(
                    dcol[64:128, :],
                    as_ar[2 * p + 1, HD:HD + 1, :].rearrange("c m -> m c"))
                nc.vector.tensor_scalar_add(dcol[:], dcol[:], 1.0)
                nc.vector.reciprocal(dcol[:], dcol[:])
                if debug:
                    nc.sync.dma_start(d_dcol[p], dcol[:])
                g2 = sm.tile([128, CIN], F16, tag="g2p" + str(p))
                nc.scalar.activation(g2[:], psg[:],
                                     mybir.ActivationFunctionType.Copy,
                                     bias=0.0, scale=dcol[:, 0:1])
                G2P.append(g2)

            if debug:
                nc.sync.dma_start(d_g20[:], G2P[0][:])
                nc.sync.dma_start(d_g21[:], G2P[1][:])

            # ---------------- dispatch + output conv ----------------
            for t in range(16):
                psy = psMID.tile([CIN, 512], F32, tag="pvy")
                nc.tensor.matmul(psy[:], G2P[0][:],
                                 MN0[:, 4 * t:4 * t + 4, :].rearrange(
                                     "p a q -> p (a q)"),
                                 start=True, stop=False)
                nc.tensor.matmul(psy[:], G2P[1][:],
                                 MN1[:, 4 * t:4 * t + 4, :].rearrange(
                                     "p a q -> p (a q)"),
                                 start=False, stop=False)
                nc.tensor.matmul(psy[:], c_bp[:], c_o512[:],
                                 start=False, stop=True)
                yst = ystg_pool.tile([CIN, 512], F32, tag="yst")
                nc.scalar.copy(yst[:], psy[:])
                nc.sync.dma_start(y_out[:, t * 512:(t + 1) * 512], yst[:])

    nc.compile()
    _CACHE[key] = nc
    return nc


def _prep_inputs(x, Wf, bf, Wv, bv, Wp, bp, sim_alpha, sim_beta):
    alpha = float(np.asarray(sim_alpha))
    beta = float(np.asarray(sim_beta))
    sgn = 1.0 if alpha >= 0 else -1.0

    wft = np.concatenate([Wf.T, bf[None, :]], 0).astype(np.float32)        # [65, 96]
    wvt = np.zeros((CIN + 1, HEADS * (HD + 1)), np.float16)
    for h in range(HEADS):
        wvt[:CIN, h * (HD + 1):h * (HD + 1) + HD] = Wv.T[:, h * HD:(h + 1) * HD]
        wvt[CIN, h * (HD + 1):h * (HD + 1) + HD] = bv[h * HD:(h + 1) * HD]
        wvt[CIN, h * (HD + 1) + HD] = 1.0
    wvt32 = Wv.T.astype(np.float32)                                        # [64, 96]
    wpt4 = np.zeros((HD, HEADS * CIN), np.float32)
    for h in range(HEADS):
        wpt4[:, h * CIN:(h + 1) * CIN] = Wp[:, h * HD:(h + 1) * HD].T
    hsel = np.zeros((COUT, HEADS), np.float32)
    for h in range(HEADS):
        hsel[h * HD:(h + 1) * HD, h] = 1.0
    abcol = np.zeros((128, 2), np.float32)
    abcol[:, 0] = abs(alpha)
    abcol[:, 1] = beta

    common = dict(
        wft=wft, wvt=wvt, wvt32=wvt32, wpt4=wpt4,
        hsel16=hsel.astype(np.float16), hsel32=hsel, hselr=hsel.T.copy(),
        bfrow=bf[None, :].astype(np.float32), bvrow=bv[None, :].astype(np.float32),
        ones64=np.ones((1, M), np.float32), ones512=np.ones((1, 512), np.float16),
        bprow16=bp[None, :].astype(np.float16),
        sgn4=np.full((HEADS, 1), sgn, np.float32), abcol=abcol,
    )

    in_maps = []
    ones_row = np.ones((1, NP), np.float32)
    for c in range(N_CORES):
        b, q = c // 4, c % 4
        xsl = x[b, :, 8 * q:8 * q + 8, :, :].reshape(CIN, NP).astype(np.float32)
        xa = np.concatenate([xsl, ones_row], 0)
        m = dict(common)
        m["xs"] = xa
        m["xs16"] = xa.astype(np.float16)
        in_maps.append(m)
    return in_maps


def kernel(x, Wf, bf, Wv, bv, Wp, bp, sim_alpha, sim_beta, _trace=False, _debug=False):
    nc = build(_debug)
    in_maps = _prep_inputs(x, Wf, bf, Wv, bv, Wp, bp, sim_alpha, sim_beta)
    res = run_bass_kernel_spmd(nc, in_maps, list(range(N_CORES)),
                               trace=_trace)
    out = np.empty((B, CIN, S, S, S), np.float32)
    for c in range(N_CORES):
        b, q = c // 4, c % 4
        out[b, :, 8 * q:8 * q + 8, :, :] = \
            res.results[c]["y_out"].reshape(CIN, 8, S, S)
    kernel._last_result = res
    return out



# revision 4
# speedup vs baseline: 1.7038x; 1.7038x over previous
"""ContextCluster (denoised) Trainium2 kernel — 8-core SPMD, v2.

Sharding: core c handles batch b=c//4, spatial w-quarter q=c%4
(8 of 32 w-planes => 8192 of 32768 points, all 4 heads).
Centers are combined across the 4 cores of each batch with an AllGather
of pooled-x; per-center aggregation sums with an AllReduce.

v2 structural changes vs v1 (357us):
 - mask transposes via PE matmuls against an fp16 identity (was: 128
   serialized DMA_TRANSPOSEs on the Sync engine = ~170us dead zone)
 - dummy AllReduce at t=0 absorbs the one-time collective barrier and
   ncfw cold-start so the real AllGather starts fast
 - value conv + per-center-sum accumulation software-pipelined into the
   sim/mask chunk loop (lag-2); x ones-row memset on chip; fp16 input
   copy dropped (value conv runs fp32 from the same X tile)
 - mask pipeline split across engines: DVE reduce_max+is_equal,
   GpSimd z/s-mults, Scalar sigmoid + psum->sbuf copies
 - transposes + dispatch scheduled in the AllReduce shadow
"""

import sys

sys.path.insert(0, '/opt/trn_rl_repo')

import numpy as np

import concourse.bass as bass
import concourse.bacc as bacc
import concourse.tile as tile
from concourse import mybir
from concourse.bass_utils import run_bass_kernel_spmd

F32 = mybir.dt.float32
F16 = mybir.dt.float16

N_CORES = 8
B, CIN, S = 2, 64, 32          # x: [B, CIN, S, S, S]
HEADS, HD = 4, 24              # heads, head_dim
COUT = HEADS * HD              # 96
PROP = 4
M = PROP ** 3                  # 64 centers
MLOC = 16                      # centers owned per core (one w-quarter)
NP = 8 * S * S                 # 8192 points per core
NCH = NP // 128                # 64 chunks of 128 points
VW = HEADS * (HD + 1)          # 100 value cols: per-head [v|1]
GROUPS = [[0, 1, 2, 3], [4, 5, 6, 7]]

_CACHE = {}


def build():
    if "nc" in _CACHE:
        return _CACHE["nc"]
    nc = bacc.Bacc("TRN2", target_bir_lowering=False, debug=False,
                   num_devices=N_CORES)

    dram_in = {}
    def din(name, shape, dt):
        dram_in[name] = nc.dram_tensor(name, shape, dt, kind="ExternalInput")
        return dram_in[name]

    xs     = din("xs",     [CIN, NP], F32)         # x slice (no ones row)
    wftP   = din("wftP",   [CIN + 1, COUT], F32)   # [Wf.T; bf]
    wvt65  = din("wvt65",  [CIN + 1, VW], F32)     # per-head [Wv_h.T|0 ; bv_h|1]
    wvt32  = din("wvt32",  [CIN, COUT], F32)       # Wv.T (for v_centers)
    wpt4   = din("wpt4",   [HD, HEADS * CIN], F32)
    hsel16 = din("hsel16", [COUT, HEADS], F16)     # head one-hot
    hsel32 = din("hsel32", [COUT, HEADS], F32)
    hselr  = din("hselr",  [HEADS, COUT], F32)     # transposed one-hot
    bvrow  = din("bvrow",  [1, COUT], F32)
    ones64 = din("ones64", [1, M], F32)
    id16   = din("id16",   [128, 128], F16)        # identity (PE transpose rhs)
    sgn4   = din("sgn4",   [HEADS, 1], F32)        # sign(alpha)
    abcol  = din("abcol",  [128, 2], F32)          # |alpha|, beta
    bpcol  = din("bpcol",  [CIN, 1], F32)          # output bias column

    y_out = nc.dram_tensor("y_out", [CIN, NP], F32, kind="ExternalOutput")

    AX = mybir.AxisListType.X
    MUL = mybir.AluOpType.mult
    with tile.TileContext(nc) as tc:
        with tc.tile_pool(name="consts", bufs=1) as consts, \
             tc.tile_pool(name="xp", bufs=1) as xp, \
             tc.tile_pool(name="fp", bufs=1) as fp, \
             tc.tile_pool(name="vp", bufs=1) as vp, \
             tc.tile_pool(name="mp", bufs=1) as mp, \
             tc.tile_pool(name="sm", bufs=1) as sm, \
             tc.tile_pool(name="ystg", bufs=3) as ystg_pool, \
             tc.tile_pool(name="psBIG", bufs=2, space="PSUM") as psBIG, \
             tc.tile_pool(name="psVT", bufs=2, space="PSUM") as psVT, \
             tc.tile_pool(name="psMISC", bufs=1, space="PSUM") as psMISC, \
             tc.tile_pool(name="psS", bufs=1, space="PSUM") as psS_pool, \
             tc.tile_pool(name="psY", bufs=2, space="PSUM") as psY_pool, \
             tc.tile_pool(name="dram", bufs=1, space="DRAM") as dram:

            # ---- dummy collective: absorbs barrier + warms ncfw ----
            dum_i = dram.tile([1, M], F32)
            dum_o = dram.tile([1, M], F32)
            nc.gpsimd.collective_compute(
                "AllReduce", mybir.AluOpType.add,
                replica_groups=GROUPS,
                ins=[dum_i.opt()], outs=[dum_o.opt()])

            # ---- input X (4 pieces) + on-chip ones row ----
            X = xp.tile([CIN + 1, NP], F32)
            nc.gpsimd.memset(X[CIN:CIN + 1, :], 1.0)
            for pc in range(4):
                nc.sync.dma_start(X[0:CIN, pc * 2048:(pc + 1) * 2048],
                                  xs[:, pc * 2048:(pc + 1) * 2048])

            # ---- const loads (scalar HWDGE queue; sync is busy with X) ----
            c_wftP = consts.tile([CIN + 1, COUT], F32)
            c_wvt = consts.tile([CIN + 1, VW], F32)
            c_wvt32 = consts.tile([CIN, COUT], F32)
            c_wpt4 = consts.tile([HD, HEADS * CIN], F32)
            c_h16 = consts.tile([COUT, HEADS], F16)
            c_h32 = consts.tile([COUT, HEADS], F32)
            c_hr = consts.tile([HEADS, COUT], F32)
            c_bv = consts.tile([1, COUT], F32)
            c_o64 = consts.tile([1, M], F32)
            c_id = consts.tile([128, 128], F16)
            c_sgn = consts.tile([HEADS, 1], F32)
            c_ab = consts.tile([128, 2], F32)
            c_bp = consts.tile([CIN, 1], F32)
            for t, d in [(c_wftP, wftP), (c_wvt, wvt65), (c_wvt32, wvt32),
                         (c_wpt4, wpt4), (c_h16, hsel16), (c_h32, hsel32),
                         (c_hr, hselr), (c_bv, bvrow), (c_o64, ones64),
                         (c_id, id16), (c_sgn, sgn4), (c_ab, abcol),
                         (c_bp, bpcol)]:
                nc.scalar.dma_start(t[:], d[:])

            # ---- pooling (per piece) -> local centers ----
            # free layout: n = w*1024 + (hb*8+h')*32 + db*8 + d'
            P1 = sm.tile([CIN, 1024], F32)
            P2 = sm.tile([CIN, 128], F32)
            for pc in range(4):
                nc.vector.reduce_sum(
                    P1[:, pc * 256:(pc + 1) * 256],
                    X[0:CIN, pc * 2048:(pc + 1) * 2048].rearrange(
                        "c (a d) -> c a d", d=8),
                    axis=AX)
                nc.vector.reduce_sum(
                    P2[:, pc * 32:(pc + 1) * 32].rearrange(
                        "c (a db) -> c a db", db=4),
                    P1[:, pc * 256:(pc + 1) * 256].rearrange(
                        "c (a hp db) -> c a db hp", hp=8, db=4),
                    axis=AX)
            PXL = sm.tile([CIN, MLOC], F32)
            nc.vector.reduce_sum(
                PXL[:].rearrange("c (hb db) -> c hb db", db=4),
                P2[:].rearrange("c (w hb db) -> c hb db w", w=8, db=4),
                axis=AX)
            nc.vector.tensor_scalar_mul(PXL[:], PXL[:], 1.0 / 512.0)

            px_loc = dram.tile([CIN, MLOC], F32)
            px_g = dram.tile([4, CIN, MLOC], F32)
            nc.sync.dma_start(px_loc[:], PXL[:])
            nc.gpsimd.collective_compute(
                "AllGather", mybir.AluOpType.bypass,
                replica_groups=GROUPS,
                ins=[px_loc.opt()], outs=[px_g.opt()])
            PXS = sm.tile([CIN + 1, M], F32)   # pooled x (all 64) + ones row
            nc.sync.dma_start(PXS[0:CIN, :].rearrange("c (r m) -> c r m", r=4),
                              px_g[:].rearrange("r c m -> c r m"))
            nc.sync.dma_start(PXS[CIN:CIN + 1, :], ones64[:])

            # ---- feat conv (fp32, NAT layout) + FSQ ----
            F = fp.tile([COUT, NP], F32)
            FSQ = fp.tile([COUT, NP], F16)
            for t in range(16):
                psf = psBIG.tile([COUT, 512], F32, tag="pfu")
                nc.tensor.matmul(psf[:], c_wftP[:], X[:, t * 512:(t + 1) * 512],
                                 start=True, stop=True)
                nc.scalar.copy(F[:, t * 512:(t + 1) * 512], psf[:])
                nc.vector.tensor_tensor(
                    out=FSQ[:, t * 512:(t + 1) * 512],
                    in0=psf[:], in1=F[:, t * 512:(t + 1) * 512], op=MUL)

            # ---- per-point sum-of-squares -> 1/||x|| ----
            psq = psMISC.tile([128, NCH * HEADS], F32, tag="misc")
            for j in range(NCH):
                nc.tensor.matmul(psq[:, j * HEADS:(j + 1) * HEADS],
                                 FSQ[:, j * 128:(j + 1) * 128],
                                 c_h16[:], start=True, stop=True)
            SQPT = sm.tile([128, NCH, HEADS], F32)
            nc.vector.tensor_copy(SQPT[:].rearrange("p a c -> p (a c)"), psq[:])
            INVPT = sm.tile([128, NCH, HEADS], F32)
            nc.scalar.sqrt(INVPT[:], SQPT[:])
            nc.vector.tensor_scalar_max(INVPT[:], INVPT[:], 1e-12)
            nc.vector.reciprocal(INVPT[:], INVPT[:])

            # ---- centers (tiny; gated on AllGather) ----
            pscf = psS_pool.tile([COUT, M], F32, tag="ps")
            nc.tensor.matmul(pscf[:], c_wftP[:], PXS[:], start=True, stop=True)
            SQC = sm.tile([COUT, M], F32)
            nc.scalar.square(SQC[:], pscf[:])
            CFS = sm.tile([COUT, M], F32)
            nc.scalar.copy(CFS[:], pscf[:])
            pshs = psS_pool.tile([HEADS, M], F32, tag="ps")
            nc.tensor.matmul(pshs[:], c_h32[:], SQC[:], start=True, stop=True)
            INV4 = sm.tile([HEADS, M], F32)
            nc.scalar.sqrt(INV4[:], pshs[:])
            nc.vector.tensor_scalar_max(INV4[:], INV4[:], 1e-12)
            nc.vector.reciprocal(INV4[:], INV4[:])
            nc.vector.tensor_scalar(out=INV4[:], in0=INV4[:],
                                    scalar1=c_sgn[:, 0:1], scalar2=None,
                                    op0=MUL)
            psie = psS_pool.tile([COUT, M], F32, tag="ps")
            nc.tensor.matmul(psie[:], c_hr[:], INV4[:], start=True, stop=True)
            CHAT = sm.tile([COUT, M], F32)
            nc.vector.tensor_tensor(out=CHAT[:], in0=CFS[:], in1=psie[:],
                                    op=MUL)
            CBLK = sm.tile([COUT, HEADS * M], F32)
            nc.gpsimd.memset(CBLK[:], 0.0)
            for h in range(HEADS):
                nc.sync.dma_start(CBLK[h * HD:(h + 1) * HD, h * M:(h + 1) * M],
                                  CHAT[h * HD:(h + 1) * HD, :])

            # v_centers (fp32): vc_h = Wv_h @ px + bv_h
            VCs = []
            for h in range(HEADS):
                psvc = psS_pool.tile([HD, M], F32, tag="ps")
                nc.tensor.matmul(psvc[:], c_wvt32[:, h * HD:(h + 1) * HD],
                                 PXS[0:CIN, :], start=True, stop=False)
                nc.tensor.matmul(psvc[:], c_bv[:, h * HD:(h + 1) * HD],
                                 c_o64[:], start=False, stop=True)
                vc = sm.tile([HD, M], F32, tag="vc" + str(h))
                nc.scalar.copy(vc[:], psvc[:])
                VCs.append(vc)

            # ---- mid loop: sim + value + mask + per-center sums ----
            Q = sm.tile([128, NCH, HEADS], F32)
            Zt = sm.tile([128, NCH, HEADS], F32)
            Ssig = sm.tile([128, NCH, HEADS], F32)
            MS = mp.tile([128, NCH, HEADS * M], F16)
            V = vp.tile([128, NCH, VW], F16)
            psas = psMISC.tile([VW, HEADS * M], F32, tag="misc")

            LAG = 2
            def emit_psas(g):
                for j in (2 * g, 2 * g + 1):
                    nc.tensor.matmul(psas[:], V[:, j, :], MS[:, j, :],
                                     start=(j == 0), stop=(j == NCH - 1))

            for g in range(32):
                j0 = 2 * g
                psu = psBIG.tile([128, 512], F32, tag="pfu")
                psv = psVT.tile([128, 2, 256], F32, tag="pvt")
                # PE: sim (argmax-critical fp32), value conv, lagged psas
                for k in range(2):
                    nc.tensor.matmul(psu[:, k * 256:(k + 1) * 256],
                                     F[:, (j0 + k) * 128:(j0 + k + 1) * 128],
                                     CBLK[:], start=True, stop=True)
                for k in range(2):
                    nc.tensor.matmul(psv[:, k, 0:VW],
                                     X[:, (j0 + k) * 128:(j0 + k + 1) * 128],
                                     c_wvt[:], start=True, stop=True)
                if g >= LAG:
                    emit_psas(g - LAG)
                # DVE: free-axis max + exact-equality mask
                nc.vector.reduce_max(
                    Q[:, j0:j0 + 2, :],
                    psu[:].rearrange("p (j h m) -> p j h m", j=2, h=HEADS),
                    axis=AX)
                nc.vector.tensor_tensor(
                    out=MS[:, j0:j0 + 2, :].rearrange(
                        "p j (h m) -> p j h m", h=HEADS),
                    in0=psu[:].rearrange("p (j h m) -> p j h m", j=2, h=HEADS),
                    in1=Q[:, j0:j0 + 2, :, None].broadcast_to(
                        [128, 2, HEADS, M]),
                    op=mybir.AluOpType.is_equal)
                # GpSimd: z = q/||x|| ; Scalar: s = sigmoid(b + a*z), V copy
                nc.gpsimd.tensor_tensor(out=Zt[:, j0:j0 + 2, :],
                                        in0=Q[:, j0:j0 + 2, :],
                                        in1=INVPT[:, j0:j0 + 2, :], op=MUL)
                nc.scalar.activation(
                    Ssig[:, j0:j0 + 2, :].rearrange("p a c -> p (a c)"),
                    Zt[:, j0:j0 + 2, :].rearrange("p a c -> p (a c)"),
                    mybir.ActivationFunctionType.Sigmoid,
                    bias=c_ab[:, 1:2], scale=c_ab[:, 0:1])
                # GpSimd: masked weights = mask * s
                nc.gpsimd.tensor_tensor(
                    out=MS[:, j0:j0 + 2, :].rearrange(
                        "p j (h m) -> p j h m", h=HEADS),
                    in0=MS[:, j0:j0 + 2, :].rearrange(
                        "p j (h m) -> p j h m", h=HEADS),
                    in1=Ssig[:, j0:j0 + 2, :, None].broadcast_to(
                        [128, 2, HEADS, M]),
                    op=MUL)
                # Scalar: value psum -> sbuf fp16
                nc.scalar.copy(V[:, j0:j0 + 2, :],
                               psv[:, :, 0:VW])
            for g in range(32 - LAG, 32):
                emit_psas(g)

            # ---- per-center sums -> AllReduce (early; sync queue is free) ----
            ASF = sm.tile([VW, HEADS * M], F32)
            nc.scalar.copy(ASF[:], psas[:])
            as_loc = dram.tile([HEADS, HD + 1, M], F32)
            as_ar = dram.tile([HEADS, HD + 1, M], F32)
            for h in range(HEADS):
                nc.sync.dma_start(
                    as_loc[h],
                    ASF[h * (HD + 1):(h + 1) * (HD + 1), h * M:(h + 1) * M])
            nc.gpsimd.collective_compute(
                "AllReduce", mybir.AluOpType.add,
                replica_groups=GROUPS,
                ins=[as_loc.opt()], outs=[as_ar.opt()])

            # ---- mask transposes on PE (fill the AllReduce shadow) ----
            MN0 = mp.tile([128, NCH, 128], F16, tag="mn0")
            MN1 = mp.tile([128, NCH, 128], F16, tag="mn1")
            for pr in range(32):
                pst = psVT.tile([128, 4, 128], F32, tag="pvt")
                for k in range(2):
                    j = 2 * pr + k
                    nc.tensor.matmul(pst[:, 2 * k, :], MS[:, j, 0:128],
                                     c_id[:], start=True, stop=True)
                    nc.tensor.matmul(pst[:, 2 * k + 1, :], MS[:, j, 128:256],
                                     c_id[:], start=True, stop=True)
                if pr % 2 == 0:
                    nc.scalar.copy(MN0[:, 2 * pr:2 * pr + 2, :],
                                   pst[:, 0::2, :])
                    nc.scalar.copy(MN1[:, 2 * pr:2 * pr + 2, :],
                                   pst[:, 1::2, :])
                else:
                    nc.vector.tensor_copy(MN0[:, 2 * pr:2 * pr + 2, :],
                                          pst[:, 0::2, :])
                    nc.vector.tensor_copy(MN1[:, 2 * pr:2 * pr + 2, :],
                                          pst[:, 1::2, :])

            # ---- agg -> G2 = d * (vsum+vc) @ WpT ----
            G2P = []
            for p in range(2):
                psg = psS_pool.tile([128, CIN], F32, tag="ps")
                for hh in range(2):
                    h = 2 * p + hh
                    ASh = sm.tile([HD + 1, M], F32, tag="ash")
                    nc.sync.dma_start(ASh[:], as_ar[h])
                    NUM = sm.tile([HD, M], F32, tag="numh")
                    nc.vector.tensor_tensor(out=NUM[:], in0=ASh[0:HD, :],
                                            in1=VCs[h][:],
                                            op=mybir.AluOpType.add)
                    nc.tensor.matmul(psg[64 * hh:64 * (hh + 1), :], NUM[:],
                                     c_wpt4[:, h * CIN:(h + 1) * CIN],
                                     start=True, stop=True)
                dcol = sm.tile([128, 1], F32, tag="dcol")
                nc.sync.dma_start(
                    dcol[0:64, :],
                    as_ar[2 * p, HD:HD + 1, :].rearrange("c m -> m c"))
                nc.sync.dma_start(
                    dcol[64:128, :],
                    as_ar[2 * p + 1, HD:HD + 1, :].rearrange("c m -> m c"))
                nc.vector.tensor_scalar_add(dcol[:], dcol[:], 1.0)
                nc.vector.reciprocal(dcol[:], dcol[:])
                g2 = sm.tile([128, CIN], F16, tag="g2p" + str(p))
                nc.scalar.activation(g2[:], psg[:],
                                     mybir.ActivationFunctionType.Copy,
                                     bias=0.0, scale=dcol[:, 0:1])
                G2P.append(g2)

            # ---- dispatch + output conv ----
            for t in range(16):
                psy = psY_pool.tile([CIN, 512], F32, tag="py")
                nc.tensor.matmul(psy[:], G2P[0][:],
                                 MN0[:, 4 * t:4 * t + 4, :].rearrange(
                                     "p a q -> p (a q)"),
                                 start=True, stop=False)
                nc.tensor.matmul(psy[:], G2P[1][:],
                                 MN1[:, 4 * t:4 * t + 4, :].rearrange(
                                     "p a q -> p (a q)"),
                                 start=False, stop=True)
                yst = ystg_pool.tile([CIN, 512], F32, tag="yst")
                if t % 2 == 0:
                    nc.scalar.activation(yst[:], psy[:],
                                         mybir.ActivationFunctionType.Identity,
                                         bias=c_bp[:, 0:1], scale=1.0)
                else:
                    nc.vector.tensor_scalar(out=yst[:], in0=psy[:],
                                            scalar1=c_bp[:, 0:1], scalar2=None,
                                            op0=mybir.AluOpType.add)
                nc.sync.dma_start(y_out[:, t * 512:(t + 1) * 512], yst[:])

    nc.compile()
    _CACHE["nc"] = nc
    return nc


def _prep_inputs(x, Wf, bf, Wv, bv, Wp, bp, sim_alpha, sim_beta):
    alpha = float(np.asarray(sim_alpha))
    beta = float(np.asarray(sim_beta))
    sgn = 1.0 if alpha >= 0 else -1.0

    wftP = np.concatenate([Wf.T, bf[None, :]], 0).astype(np.float32)  # [65, 96]
    wvt65 = np.zeros((CIN + 1, VW), np.float32)
    for h in range(HEADS):
        wvt65[:CIN, h * (HD + 1):h * (HD + 1) + HD] = Wv.T[:, h * HD:(h + 1) * HD]
        wvt65[CIN, h * (HD + 1):h * (HD + 1) + HD] = bv[h * HD:(h + 1) * HD]
        wvt65[CIN, h * (HD + 1) + HD] = 1.0
    wvt32 = Wv.T.astype(np.float32)                                   # [64, 96]
    wpt4 = np.zeros((HD, HEADS * CIN), np.float32)
    for h in range(HEADS):
        wpt4[:, h * CIN:(h + 1) * CIN] = Wp[:, h * HD:(h + 1) * HD].T
    hsel = np.zeros((COUT, HEADS), np.float32)
    for h in range(HEADS):
        hsel[h * HD:(h + 1) * HD, h] = 1.0
    abcol = np.zeros((128, 2), np.float32)
    abcol[:, 0] = abs(alpha)
    abcol[:, 1] = beta

    common = dict(
        wftP=wftP, wvt65=wvt65, wvt32=wvt32, wpt4=wpt4,
        hsel16=hsel.astype(np.float16), hsel32=hsel, hselr=hsel.T.copy(),
        bvrow=bv[None, :].astype(np.float32),
        ones64=np.ones((1, M), np.float32),
        id16=np.eye(128, dtype=np.float16),
        sgn4=np.full((HEADS, 1), sgn, np.float32), abcol=abcol,
        bpcol=bp[:, None].astype(np.float32),
    )

    in_maps = []
    for c in range(N_CORES):
        b, q = c // 4, c % 4
        m = dict(common)
        m["xs"] = x[b, :, 8 * q:8 * q + 8, :, :].reshape(CIN, NP).astype(
            np.float32)
        in_maps.append(m)
    return in_maps


def kernel(x, Wf, bf, Wv, bv, Wp, bp, sim_alpha, sim_beta, _trace=False):
    nc = build()
    in_maps = _prep_inputs(x, Wf, bf, Wv, bv, Wp, bp, sim_alpha, sim_beta)
    res = run_bass_kernel_spmd(nc, in_maps, list(range(N_CORES)),
                               trace=_trace)
    out = np.empty((B, CIN, S, S, S), np.float32)
    for c in range(N_CORES):
        b, q = c // 4, c % 4
        out[b, :, 8 * q:8 * q + 8, :, :] = \
            res.results[c]["y_out"].reshape(CIN, 8, S, S)
    kernel._last_result = res
    return out


# revision 10
# speedup vs baseline: 1.9340x; 1.1351x over previous
"""ContextCluster (denoised) Trainium2 kernel — 8-core SPMD, v3.

Sharding: core c handles batch b=c//4, spatial w-quarter q=c%4
(8 of 32 w-planes => 8192 of 32768 points, all 4 heads).
Centers are combined across the 4 cores of each batch with an AllGather
of pooled-x; per-center aggregation sums with an AllReduce.

v3 vs v2 (210us):
 - feat + sim matmuls stream as float32r (1 cyc/row at >=256 moving
   cols vs fp32's 4) via AP bitcast; psum/compare stay fp32-exact
   within the kernel (argmax self-consistent)
 - value conv in fp16 from an on-chip cast copy (fp32 LOW_HIGH value
   conv was ~1.1us/chunk of PE in v2)
 - mask transposes via PE matmuls vs fp16 identity, software-pipelined
   (lag 2) into the sim loop; last groups deferred into the AllReduce
   shadow; MN stored as one [128,NCH,2,128] tile so each group needs a
   single psum->sbuf copy
 - engine split: DVE max+is_equal, GpSimd z/s-mults, Scalar sigmoid +
   V/MN copies; no dummy collective (the runtime barrier self-starts
   at ~21us regardless; an extra collective only queues ahead of the
   real AllGather)
"""

import sys

sys.path.insert(0, '/opt/trn_rl_repo')

import numpy as np

import concourse.bass as bass
import concourse.bacc as bacc
import concourse.tile as tile
from concourse import mybir
from concourse.bass_utils import run_bass_kernel_spmd

F32 = mybir.dt.float32
F32R = mybir.dt.float32r
F16 = mybir.dt.float16

N_CORES = 8
B, CIN, S = 2, 64, 32          # x: [B, CIN, S, S, S]
HEADS, HD = 4, 24              # heads, head_dim
COUT = HEADS * HD              # 96
PROP = 4
M = PROP ** 3                  # 64 centers
MLOC = 16                      # centers owned per core (one w-quarter)
NP = 8 * S * S                 # 8192 points per core
NCH = NP // 128                # 64 chunks of 128 points
VW = HEADS * (HD + 1)          # 100 value cols: per-head [v|1]
GROUPS = [[0, 1, 2, 3], [4, 5, 6, 7]]

_CACHE = {}


def build():
    if "nc" in _CACHE:
        return _CACHE["nc"]
    nc = bacc.Bacc("TRN2", target_bir_lowering=False, debug=False,
                   num_devices=N_CORES)

    dram_in = {}
    def din(name, shape, dt):
        dram_in[name] = nc.dram_tensor(name, shape, dt, kind="ExternalInput")
        return dram_in[name]

    xs     = din("xs",     [CIN, NP], F32)         # x slice
    wftP   = din("wftP",   [CIN + 1, COUT], F32)   # [Wf.T; bf]
    wvt16  = din("wvt16",  [CIN + 1, VW], F16)     # per-head [Wv_h.T|0 ; bv_h|1]
    wvt32  = din("wvt32",  [CIN, COUT], F32)       # Wv.T (for v_centers)
    wpt4   = din("wpt4",   [HD, HEADS * CIN], F32)
    hsel16 = din("hsel16", [COUT, HEADS], F16)     # head one-hot
    hsel32 = din("hsel32", [COUT, HEADS], F32)
    hselr  = din("hselr",  [HEADS, COUT], F32)     # transposed one-hot
    bvrow  = din("bvrow",  [1, COUT], F32)
    bfcol  = din("bfcol",  [COUT, 1], F32)         # feat bias column
    ones64 = din("ones64", [1, M], F32)
    ones8k = din("ones8k", [1, NP], F16)           # X16 bias row
    id16   = din("id16",   [128, 128], F16)        # identity (PE transpose rhs)
    sgn4   = din("sgn4",   [HEADS, 1], F32)        # sign(alpha)
    abcol  = din("abcol",  [128, 2], F32)          # |alpha|, beta
    bpcol  = din("bpcol",  [CIN, 1], F32)          # output bias column

    y_out = nc.dram_tensor("y_out", [CIN, NP], F32, kind="ExternalOutput")

    AX = mybir.AxisListType.X
    MUL = mybir.AluOpType.mult
    with tile.TileContext(nc) as tc:
        with tc.tile_pool(name="consts", bufs=1) as consts, \
             tc.tile_pool(name="xp", bufs=1) as xp, \
             tc.tile_pool(name="fp", bufs=1) as fp, \
             tc.tile_pool(name="vp", bufs=1) as vp, \
             tc.tile_pool(name="mp", bufs=1) as mp, \
             tc.tile_pool(name="sm", bufs=1) as sm, \
             tc.tile_pool(name="ystg", bufs=3) as ystg_pool, \
             tc.tile_pool(name="psBIG", bufs=3, space="PSUM") as psBIG, \
             tc.tile_pool(name="psVT", bufs=3, space="PSUM") as psVT, \
             tc.tile_pool(name="psAS", bufs=1, space="PSUM") as psAS, \
             tc.tile_pool(name="psSM", bufs=1, space="PSUM") as psSM, \
             tc.tile_pool(name="dram", bufs=1, space="DRAM") as dram:

            # ---- input X (4 pieces) ----
            X = xp.tile([CIN, NP], F32)
            for pc in range(4):
                nc.sync.dma_start(X[:, pc * 2048:(pc + 1) * 2048],
                                  xs[:, pc * 2048:(pc + 1) * 2048])

            # ---- const loads (scalar HWDGE queue; sync is busy with X) ----
            c_wftP = consts.tile([CIN + 1, COUT], F32)
            c_wvt16 = consts.tile([CIN + 1, VW], F16)
            c_wvt32 = consts.tile([CIN, COUT], F32)
            c_wpt4 = consts.tile([HD, HEADS * CIN], F32)
            c_h16 = consts.tile([COUT, HEADS], F16)
            c_h32 = consts.tile([COUT, HEADS], F32)
            c_hr = consts.tile([HEADS, COUT], F32)
            c_bv = consts.tile([1, COUT], F32)
            c_bf = consts.tile([COUT, 1], F32)
            c_o64 = consts.tile([1, M], F32)
            c_id = consts.tile([128, 128], F16)
            c_sgn = consts.tile([HEADS, 1], F32)
            c_ab = consts.tile([128, 2], F32)
            c_bp = consts.tile([CIN, 1], F32)
            for t, d in [(c_wftP, wftP), (c_wvt16, wvt16), (c_wvt32, wvt32),
                         (c_wpt4, wpt4), (c_h16, hsel16), (c_h32, hsel32),
                         (c_hr, hselr), (c_bv, bvrow), (c_bf, bfcol),
                         (c_o64, ones64), (c_id, id16), (c_sgn, sgn4),
                         (c_ab, abcol), (c_bp, bpcol)]:
                nc.scalar.dma_start(t[:], d[:])

            # ---- X16 (fp16 copy for the value conv) ----
            X16 = xp.tile([CIN + 1, NP], F16)
            nc.scalar.dma_start(X16[CIN:CIN + 1, :], ones8k[:])
            for pc in range(4):
                nc.vector.tensor_copy(X16[0:CIN, pc * 2048:(pc + 1) * 2048],
                                      X[:, pc * 2048:(pc + 1) * 2048])

            # ---- pooling (per piece) -> local centers ----
            # free layout: n = w*1024 + (hb*8+h')*32 + db*8 + d'
            P1 = sm.tile([CIN, 1024], F32)
            P2 = sm.tile([CIN, 128], F32)
            for pc in range(4):
                nc.vector.reduce_sum(
                    P1[:, pc * 256:(pc + 1) * 256],
                    X[:, pc * 2048:(pc + 1) * 2048].rearrange(
                        "c (a d) -> c a d", d=8),
                    axis=AX)
                nc.vector.reduce_sum(
                    P2[:, pc * 32:(pc + 1) * 32].rearrange(
                        "c (a db) -> c a db", db=4),
                    P1[:, pc * 256:(pc + 1) * 256].rearrange(
                        "c (a hp db) -> c a db hp", hp=8, db=4),
                    axis=AX)
            PXL = sm.tile([CIN, MLOC], F32)
            nc.vector.reduce_sum(
                PXL[:].rearrange("c (hb db) -> c hb db", db=4),
                P2[:].rearrange("c (w hb db) -> c hb db w", w=8, db=4),
                axis=AX)
            nc.vector.tensor_scalar_mul(PXL[:], PXL[:], 1.0 / 512.0)

            px_loc = dram.tile([CIN, MLOC], F32)
            px_g = dram.tile([4, CIN, MLOC], F32)
            nc.sync.dma_start(px_loc[:], PXL[:])
            nc.gpsimd.collective_compute(
                "AllGather", mybir.AluOpType.bypass,
                replica_groups=GROUPS,
                ins=[px_loc.opt()], outs=[px_g.opt()])
            PXS = sm.tile([CIN + 1, M], F32)   # pooled x (all 64) + ones row
            nc.sync.dma_start(PXS[0:CIN, :].rearrange("c (r m) -> c r m", r=4),
                              px_g[:].rearrange("r c m -> c r m"))
            nc.sync.dma_start(PXS[CIN:CIN + 1, :], ones64[:])

            # ---- feat conv (fp32; F stored rounded-f32r for the sim) ----
            F = fp.tile([COUT, NP], F32R)
            FSQ = fp.tile([COUT, NP], F16)
            for t in range(16):
                psf = psBIG.tile([COUT, 512], F32, tag="pfu")
                nc.tensor.matmul(psf[:],
                                 c_wftP[0:CIN, :],
                                 X[:, t * 512:(t + 1) * 512],
                                 start=True, stop=True)
                nc.scalar.activation(F[:, t * 512:(t + 1) * 512], psf[:],
                                     mybir.ActivationFunctionType.Identity,
                                     bias=c_bf[:, 0:1], scale=1.0)
                nc.vector.tensor_tensor(
                    out=FSQ[:, t * 512:(t + 1) * 512],
                    in0=F[:, t * 512:(t + 1) * 512].bitcast(F32),
                    in1=F[:, t * 512:(t + 1) * 512].bitcast(F32), op=MUL)

            # ---- value conv (fp16) ----
            V = vp.tile([128, NCH, VW], F16)
            for i in range(16):
                psv = psVT.tile([128, 4, VW], F32, tag="pvt")
                for k in range(4):
                    j = 4 * i + k
                    nc.tensor.matmul(psv[:, k, :],
                                     X16[:, j * 128:(j + 1) * 128],
                                     c_wvt16[:], start=True, stop=True)
                nc.scalar.copy(V[:, 4 * i:4 * i + 4, :], psv[:])

            # ---- per-point sum-of-squares -> 1/||x|| ----
            psq = psSM.tile([128, NCH * HEADS], F32, tag="sm")
            for j in range(NCH):
                nc.tensor.matmul(psq[:, j * HEADS:(j + 1) * HEADS],
                                 FSQ[:, j * 128:(j + 1) * 128],
                                 c_h16[:], start=True, stop=True)
            SQPT = sm.tile([128, NCH, HEADS], F32)
            nc.vector.tensor_copy(SQPT[:].rearrange("p a c -> p (a c)"), psq[:])
            INVPT = sm.tile([128, NCH, HEADS], F32)
            nc.scalar.sqrt(INVPT[:], SQPT[:])
            nc.vector.tensor_scalar_max(INVPT[:], INVPT[:], 1e-12)
            nc.vector.reciprocal(INVPT[:], INVPT[:])

            # ---- centers (tiny; gated on AllGather) ----
            pscf = psSM.tile([COUT, M], F32, tag="sm")
            nc.tensor.matmul(pscf[:], c_wftP[:], PXS[:], start=True, stop=True)
            SQC = sm.tile([COUT, M], F32)
            nc.scalar.square(SQC[:], pscf[:])
            CFS = sm.tile([COUT, M], F32)
            nc.scalar.copy(CFS[:], pscf[:])
            pshs = psSM.tile([HEADS, M], F32, tag="sm")
            nc.tensor.matmul(pshs[:], c_h32[:], SQC[:], start=True, stop=True)
            INV4 = sm.tile([HEADS, M], F32)
            nc.scalar.sqrt(INV4[:], pshs[:])
            nc.vector.tensor_scalar_max(INV4[:], INV4[:], 1e-12)
            nc.vector.reciprocal(INV4[:], INV4[:])
            nc.vector.tensor_scalar(out=INV4[:], in0=INV4[:],
                                    scalar1=c_sgn[:, 0:1], scalar2=None,
                                    op0=MUL)
            psie = psSM.tile([COUT, M], F32, tag="sm")
            nc.tensor.matmul(psie[:], c_hr[:], INV4[:], start=True, stop=True)
            CHAT = sm.tile([COUT, M], F32)
            nc.vector.tensor_tensor(out=CHAT[:], in0=CFS[:], in1=psie[:],
                                    op=MUL)
            CBLK32 = sm.tile([COUT, HEADS * M], F32)
            nc.gpsimd.memset(CBLK32[:], 0.0)
            for h in range(HEADS):
                nc.sync.dma_start(
                    CBLK32[h * HD:(h + 1) * HD, h * M:(h + 1) * M],
                    CHAT[h * HD:(h + 1) * HD, :])
            CBLK = sm.tile([COUT, HEADS * M], F32R)
            nc.vector.tensor_copy(CBLK[:], CBLK32[:])

            # v_centers (fp32): vc_h = Wv_h @ px + bv_h
            VCs = []
            for h in range(HEADS):
                psvc = psSM.tile([HD, M], F32, tag="sm")
                nc.tensor.matmul(psvc[:], c_wvt32[:, h * HD:(h + 1) * HD],
                                 PXS[0:CIN, :], start=True, stop=False)
                nc.tensor.matmul(psvc[:], c_bv[:, h * HD:(h + 1) * HD],
                                 c_o64[:], start=False, stop=True)
                vc = sm.tile([HD, M], F32, tag="vc" + str(h))
                nc.scalar.copy(vc[:], psvc[:])
                VCs.append(vc)

            # ---- mid loop: sim + mask + per-center sums + transposes ----
            Q = sm.tile([128, NCH, HEADS], F32)
            Zt = sm.tile([128, NCH, HEADS], F32)
            Ssig = sm.tile([128, NCH, HEADS], F32)
            MS = mp.tile([128, NCH, HEADS * M], F16)
            MN = mp.tile([128, NCH, 2, 128], F16)
            psas = psAS.tile([VW, HEADS * M], F32, tag="as")

            LAG = 2
            DEFER = 6   # last DEFER groups' transposes go after the AR trigger

            def emit_psas(g):
                for j in (2 * g, 2 * g + 1):
                    nc.tensor.matmul(psas[:], V[:, j, :], MS[:, j, :],
                                     start=(j == 0), stop=(j == NCH - 1))

            def emit_transp(g):
                pst = psVT.tile([128, 4, 128], F32, tag="pvt")
                for k in range(2):
                    j = 2 * g + k
                    nc.tensor.matmul(pst[:, 2 * k, :], MS[:, j, 0:128],
                                     c_id[:], start=True, stop=True)
                    nc.tensor.matmul(pst[:, 2 * k + 1, :], MS[:, j, 128:256],
                                     c_id[:], start=True, stop=True)
                nc.scalar.copy(MN[:, 2 * g:2 * g + 2, :, :], pst[:])

            for g in range(32):
                j0 = 2 * g
                psu = psBIG.tile([128, 512], F32, tag="pfu")
                for k in range(2):
                    nc.tensor.matmul(
                        psu[:, k * 256:(k + 1) * 256],
                        F[:, (j0 + k) * 128:(j0 + k + 1) * 128],
                        CBLK[:], start=True, stop=True)
                if g >= LAG:
                    emit_psas(g - LAG)
                    if g - LAG < 32 - DEFER:
                        emit_transp(g - LAG)
                # DVE: free-axis max + exact-equality mask
                nc.vector.reduce_max(
                    Q[:, j0:j0 + 2, :],
                    psu[:].rearrange("p (j h m) -> p j h m", j=2, h=HEADS),
                    axis=AX)
                nc.vector.tensor_tensor(
                    out=MS[:, j0:j0 + 2, :].rearrange(
                        "p j (h m) -> p j h m", h=HEADS),
                    in0=psu[:].rearrange("p (j h m) -> p j h m", j=2, h=HEADS),
                    in1=Q[:, j0:j0 + 2, :, None].broadcast_to(
                        [128, 2, HEADS, M]),
                    op=mybir.AluOpType.is_equal)
                # GpSimd: z = q/||x|| ; Scalar: s = sigmoid(b + a*z)
                nc.gpsimd.tensor_tensor(out=Zt[:, j0:j0 + 2, :],
                                        in0=Q[:, j0:j0 + 2, :],
                                        in1=INVPT[:, j0:j0 + 2, :], op=MUL)
                nc.scalar.activation(
                    Ssig[:, j0:j0 + 2, :].rearrange("p a c -> p (a c)"),
                    Zt[:, j0:j0 + 2, :].rearrange("p a c -> p (a c)"),
                    mybir.ActivationFunctionType.Sigmoid,
                    bias=c_ab[:, 1:2], scale=c_ab[:, 0:1])
                # GpSimd: masked weights = mask * s
                nc.gpsimd.tensor_tensor(
                    out=MS[:, j0:j0 + 2, :].rearrange(
                        "p j (h m) -> p j h m", h=HEADS),
                    in0=MS[:, j0:j0 + 2, :].rearrange(
                        "p j (h m) -> p j h m", h=HEADS),
                    in1=Ssig[:, j0:j0 + 2, :, None].broadcast_to(
                        [128, 2, HEADS, M]),
                    op=MUL)
            for g in range(32 - LAG, 32):
                emit_psas(g)

            # ---- per-center sums -> AllReduce ----
            ASF = sm.tile([VW, HEADS * M], F32)
            nc.scalar.copy(ASF[:], psas[:])
            as_loc = dram.tile([HEADS, HD + 1, M], F32)
            as_ar = dram.tile([HEADS, HD + 1, M], F32)
            for h in range(HEADS):
                nc.sync.dma_start(
                    as_loc[h],
                    ASF[h * (HD + 1):(h + 1) * (HD + 1), h * M:(h + 1) * M])
            nc.gpsimd.collective_compute(
                "AllReduce", mybir.AluOpType.add,
                replica_groups=GROUPS,
                ins=[as_loc.opt()], outs=[as_ar.opt()])

            # deferred transposes fill the AllReduce shadow
            for g in range(32 - DEFER, 32):
                emit_transp(g)

            # ---- agg -> G2 = d * (vsum+vc) @ WpT ----
            G2P = []
            for p in range(2):
                psg = psSM.tile([128, CIN], F32, tag="sm")
                for hh in range(2):
                    h = 2 * p + hh
                    ASh = sm.tile([HD + 1, M], F32, tag="ash")
                    nc.sync.dma_start(ASh[:], as_ar[h])
                    NUM = sm.tile([HD, M], F32, tag="numh")
                    nc.vector.tensor_tensor(out=NUM[:], in0=ASh[0:HD, :],
                                            in1=VCs[h][:],
                                            op=mybir.AluOpType.add)
                    nc.tensor.matmul(psg[64 * hh:64 * (hh + 1), :], NUM[:],
                                     c_wpt4[:, h * CIN:(h + 1) * CIN],
                                     start=True, stop=True)
                dcol = sm.tile([128, 1], F32, tag="dcol")
                nc.sync.dma_start(
                    dcol[0:64, :],
                    as_ar[2 * p, HD:HD + 1, :].rearrange("c m -> m c"))
                nc.sync.dma_start(
                    dcol[64:128, :],
                    as_ar[2 * p + 1, HD:HD + 1, :].rearrange("c m -> m c"))
                nc.vector.tensor_scalar_add(dcol[:], dcol[:], 1.0)
                nc.vector.reciprocal(dcol[:], dcol[:])
                g2 = sm.tile([128, CIN], F16, tag="g2p" + str(p))
                nc.scalar.activation(g2[:], psg[:],
                                     mybir.ActivationFunctionType.Copy,
                                     bias=0.0, scale=dcol[:, 0:1])
                G2P.append(g2)

            # ---- dispatch + output conv ----
            for t in range(16):
                psy = psBIG.tile([CIN, 512], F32, tag="pfu")
                nc.tensor.matmul(psy[:], G2P[0][:],
                                 MN[:, 4 * t:4 * t + 4, 0, :],
                                 start=True, stop=False)
                nc.tensor.matmul(psy[:], G2P[1][:],
                                 MN[:, 4 * t:4 * t + 4, 1, :],
                                 start=False, stop=True)
                yst = ystg_pool.tile([CIN, 512], F32, tag="yst")
                nc.scalar.activation(yst[:], psy[:],
                                     mybir.ActivationFunctionType.Identity,
                                     bias=c_bp[:, 0:1], scale=1.0)
                nc.sync.dma_start(y_out[:, t * 512:(t + 1) * 512], yst[:])

    nc.compile()
    _CACHE["nc"] = nc
    return nc


def _prep_inputs(x, Wf, bf, Wv, bv, Wp, bp, sim_alpha, sim_beta):
    alpha = float(np.asarray(sim_alpha))
    beta = float(np.asarray(sim_beta))
    sgn = 1.0 if alpha >= 0 else -1.0

    wftP = np.concatenate([Wf.T, bf[None, :]], 0).astype(np.float32)  # [65, 96]
    wvt = np.zeros((CIN + 1, VW), np.float16)
    for h in range(HEADS):
        wvt[:CIN, h * (HD + 1):h * (HD + 1) + HD] = \
            Wv.T[:, h * HD:(h + 1) * HD].astype(np.float16)
        wvt[CIN, h * (HD + 1):h * (HD + 1) + HD] = bv[h * HD:(h + 1) * HD]
        wvt[CIN, h * (HD + 1) + HD] = 1.0
    wvt32 = Wv.T.astype(np.float32)                                   # [64, 96]
    wpt4 = np.zeros((HD, HEADS * CIN), np.float32)
    for h in range(HEADS):
        wpt4[:, h * CIN:(h + 1) * CIN] = Wp[:, h * HD:(h + 1) * HD].T
    hsel = np.zeros((COUT, HEADS), np.float32)
    for h in range(HEADS):
        hsel[h * HD:(h + 1) * HD, h] = 1.0
    abcol = np.zeros((128, 2), np.float32)
    abcol[:, 0] = abs(alpha)
    abcol[:, 1] = beta

    common = dict(
        wftP=wftP, wvt16=wvt, wvt32=wvt32, wpt4=wpt4,
        hsel16=hsel.astype(np.float16), hsel32=hsel, hselr=hsel.T.copy(),
        bvrow=bv[None, :].astype(np.float32),
        bfcol=bf[:, None].astype(np.float32),
        ones64=np.ones((1, M), np.float32),
        ones8k=np.ones((1, NP), np.float16),
        id16=np.eye(128, dtype=np.float16),
        sgn4=np.full((HEADS, 1), sgn, np.float32), abcol=abcol,
        bpcol=bp[:, None].astype(np.float32),
    )

    in_maps = []
    for c in range(N_CORES):
        b, q = c // 4, c % 4
        m = dict(common)
        m["xs"] = x[b, :, 8 * q:8 * q + 8, :, :].reshape(CIN, NP).astype(
            np.float32)
        in_maps.append(m)
    return in_maps


def kernel(x, Wf, bf, Wv, bv, Wp, bp, sim_alpha, sim_beta, _trace=False):
    nc = build()
    in_maps = _prep_inputs(x, Wf, bf, Wv, bv, Wp, bp, sim_alpha, sim_beta)
    res = run_bass_kernel_spmd(nc, in_maps, list(range(N_CORES)),
                               trace=_trace)
    out = np.empty((B, CIN, S, S, S), np.float32)
    for c in range(N_CORES):
        b, q = c // 4, c % 4
        out[b, :, 8 * q:8 * q + 8, :, :] = \
            res.results[c]["y_out"].reshape(CIN, 8, S, S)
    kernel._last_result = res
    return out
